# revision 67
# baseline (speedup 1.0000x reference)
"""Trainium2 Bass kernel for policy-masked attention (nn_Attention_5007931867377).

Reference computation (per batch b):
    qkv = x @ w_qkv.T ; split into q,k,v heads [H=6, N=1568, D=64]
    s   = (q @ k.T) * D**-0.5
    mask[m] visibility per key + diagonal always kept
    e   = exp(s - max) * mask ; attn = (e + EPS/N)/(sum e + EPS)
    out = (attn @ v) concat heads @ w_proj.T + b_proj

Strategy: pure data parallel, one batch element per NeuronCore (8 cores).
Per-core dataflow (everything transposed so softmax reductions are on the
free axis and e^T feeds the V-matmul without on-chip transposes):
  - host passes x^T (bf16); on chip: Q^T,K^T in [c_out, n] layout, V in [n, d]
  - tokens permuted host-side so kept keys come first; KC chunks of 128
    keys run the full path, chunks BSTART.. are dropped keys, visible only
    to their own query (diagonal)
  - scores^T[m, q] = K^T.T @ Q^T  (PSUM, per 128-key chunk x query piece)
  - key-mask folded into exp as per-partition bias (-30 * (1-p))
  - diagonal-keep for dropped keys: per-head band tiles batch FOUR 128x128
    diagonal blocks into one PSUM bank; one 512-wide exp (no bias), then a
    DVE mul with a host-built diag(1-p) mask keeps exactly the wanted
    diagonal entries.  qk is padded to 13*128 columns (zeroed) so the last
    chunk is uniform.  Band vmms run before the last kept chunk so kept
    chunk KC-1 uniformly carries the accumulation stop.
  - e^T = exp(0.125*s + bias) ACT -> SBUF (bf16)
  - V augmented with a ones column: outT_aug[65, q] = V_aug.T @ e^T gives
    attention output rows 0..63 and the softmax denominator in row 64
  - normalize: DVE copy of the denominator row, gpsimd partition_broadcast,
    DVE reciprocal_approx_fast, DVE multiply (all-SBUF except the copy;
    custom-DVE ops must NOT read PSUM directly - wrong results on HW)
  - proj: y[n, :] = oT.T @ w_proj.T (+ bias via K=1 ones matmul), DVE copy
    to bf16, DMA out bf16 (host upcasts).

Schedule: xt/w_qkv DMAs are column-split so the first scores' inputs land
first; only qkv cc0/cc3 and V chunks 0..5 run up front, V chunks 6.. and
qkv cc1/4 (heads 2,3) / cc2/5 (heads 4,5) interleave into the attention
stream as PSUM-slot units filling exp-wait gaps.

All matmuls bf16: float32r matmuls don't register as PE activity for
the HAM clock gate and their 4-byte fused weight loads serialize; bf16
runs warm at 2.4 GHz. Max-subtraction is dropped: scores ~ N(0,1).
"""

import sys

if "/opt/trn_rl_repo" not in sys.path:
    sys.path.insert(0, "/opt/trn_rl_repo")

import numpy as np

B, N, C, H = 8, 1568, 384, 6
D = C // H  # 64
SCALE = D ** -0.5  # 0.125
EPS = 1e-6
NEG = -30.0  # masked-key exp bias (exp(-30) ~ 9e-14, way below EPS/N)

P = 128
NCH = (N + P - 1) // P  # 13 key/token chunks (12 x 128 + 1 x 32)
CHS = [min(P, N - i * P) for i in range(NCH)]
NPAD = NCH * P  # 1664: qk padded so band chunks are uniformly 128 wide
# query pieces: 128-aligned, <=1 PSUM bank each
QPIECES = [(0, 512), (512, 512), (1024, 512), (1536, 32)]
SLOTW = 512
NPIECE = len(QPIECES)


def _subsplit(qo, qw, step=512):
    return [(qo + o, min(step, qw - o)) for o in range(0, qw, step)]


def _piece_of_chunk(j):
    for pi, (qo, qw) in enumerate(QPIECES):
        if qo <= j * P and j * P + CHS[j] <= qo + qw:
            return pi
    raise AssertionError


_CACHE = {}


def _build_nc(KC, BSTART, HASB):
    import concourse.tile as tile
    from concourse import bacc, mybir

    dt = mybir.dt
    f32 = dt.float32
    bf16 = dt.bfloat16
    AF = mybir.ActivationFunctionType

    NB = NCH - BSTART  # number of diagonal band chunks
    BGROUPS = []  # band chunk groups: 4 blocks fill one PSUM bank
    _b = list(range(BSTART, NCH))
    for g0 in range(0, NB, 4):
        BGROUPS.append(_b[g0 : g0 + 4])

    nc = bacc.Bacc()

    xT_d = nc.declare_dram_parameter("xT", [C, N], bf16, isOutput=False)
    wqkvT_d = nc.declare_dram_parameter("wqkvT", [C, 3 * C], bf16, isOutput=False)
    wprojT_d = nc.declare_dram_parameter("wprojT", [C, C], bf16, isOutput=False)
    bias_d = nc.declare_dram_parameter("bias_exp", [P, NCH], f32, isOutput=False)
    dmask_d = nc.declare_dram_parameter("dmask", [P, NCH, P], bf16, isOutput=False)
    bvec_d = nc.declare_dram_parameter("bvec", [1, C], bf16, isOutput=False)
    out_d = nc.declare_dram_parameter("out", [N, C], bf16, isOutput=True)

    with tile.TileContext(nc, pool_alloc_mode="queue") as tc:
        with (
            tc.tile_pool(name="persist", bufs=1) as pp,
            tc.tile_pool(name="work", bufs=6) as wp,
        ):
            # ---- persistent SBUF tensors ----
            xt = pp.tile([P, 3, N], bf16, tag="xt")  # x^T chunks (c rows)
            wqkv = pp.tile([P, 3, 3 * C], bf16, tag="wqkv")
            wproj = pp.tile([P, 3, C], bf16, tag="wproj")
            qk = pp.tile([P, 6, NPAD], bf16, tag="qk")  # Q^T(0..2), K^T(3..5)
            vaug = pp.tile([P, NCH, H, D + 1], bf16, tag="vaug")
            ot = pp.tile([P, 3, N], bf16, tag="ot")  # normalized attn out ^T
            bias = pp.tile([P, NCH], f32, tag="bias")
            dmask = pp.tile([P, NB, P], bf16, tag="dmask")
            bvec = pp.tile([1, C], bf16, tag="bvec")
            ones = pp.tile([1, P], bf16, tag="ones")
            warm = pp.tile([1, 1], f32, tag="warm")

            # DMA order follows the first scores' dependency chain
            xr = xT_d[:].rearrange("(a p) n -> p a n", p=P)
            qr_ = wqkvT_d[:].rearrange("(a p) n -> p a n", p=P)
            pr = wprojT_d[:].rearrange("(a p) n -> p a n", p=P)
            for po, pw in ((0, 512), (512, 512)):
                for c in range(3):
                    nc.sync.dma_start(
                        xt[:, c, po : po + pw], xr[:, c, po : po + pw]
                    )
            for o, w in ((0, P), (3 * P, P)):  # w_qkv cc0, cc3 columns
                for c in range(3):
                    nc.sync.dma_start(
                        wqkv[:, c, o : o + w], qr_[:, c, o : o + w]
                    )
            for c in range(3):
                nc.sync.dma_start(xt[:, c, 1024:], xr[:, c, 1024:])
            for c in range(3):  # V columns
                nc.sync.dma_start(
                    wqkv[:, c, 2 * C : 3 * C], qr_[:, c, 2 * C : 3 * C]
                )
            nc.sync.dma_start(bias[:, :], bias_d[:])
            for o, w in ((P, 2 * P), (4 * P, 2 * P)):  # cc1/2, cc4/5
                for c in range(3):
                    nc.sync.dma_start(
                        wqkv[:, c, o : o + w], qr_[:, c, o : o + w]
                    )
            nc.sync.dma_start(dmask[:, :, :], dmask_d[:, BSTART:NCH, :])
            for c in range(3):
                nc.sync.dma_start(wproj[:, c, :], pr[:, c, :])
            nc.sync.dma_start(bvec[:, :], bvec_d[:])
            nc.vector.memset(ones[:, :], 1.0)
            nc.vector.memset(warm[:, :], NEG)
            # dummy exp so the ACT table set loads during the DMA prologue
            nc.scalar.activation(warm[:, :], warm[:, :], AF.Exp)
            # zero the pad: band chunk NCH-1 runs 128 wide (pad scores
            # exp(0)=1 are killed by dmask; vaug pad rows contribute 0)
            nc.vector.memset(qk[:, :, N:NPAD], 0.0)
            nc.vector.memset(vaug[:, NCH - 1, :, :], 0.0)

            # ---- phase 1: minimal upfront qkv ----
            def _qkv_unit(pool, tag, cc, qo, qw):
                def emit():
                    ps = pool.tile([P, 512], f32, tag=tag,
                                   name=f"qp{cc}_{qo}")
                    for c in range(3):
                        nc.tensor.matmul(
                            ps[:, :qw],
                            wqkv[:, c, cc * P : (cc + 1) * P],
                            xt[:, c, qo : qo + qw],
                            start=(c == 0),
                            stop=(c == 2),
                        )
                    nc.vector.tensor_copy(qk[:, cc, qo : qo + qw], ps[:, :qw])
                return emit

            def _v_unit(pool, i):
                def emit():
                    m = CHS[i]
                    ps = pool.tile([P, 512], f32, tag="sc", name=f"v{i}")
                    for c in range(3):
                        nc.tensor.matmul(
                            ps[:m, :C],
                            xt[:, c, i * P : i * P + m],
                            wqkv[:, c, 2 * C : 3 * C],
                            start=(c == 0),
                            stop=(c == 2),
                        )
                    nc.vector.tensor_copy(
                        vaug[:m, i, :, 0:D],
                        ps[:m, :C].rearrange("p (h d) -> p h d", h=H),
                    )
                    nc.vector.memset(vaug[:m, i, :, D : D + 1], 1.0)
                return emit

            NV_UP = min(3, NCH)  # V chunks built before the task stream

            # ---- phase 2: attention, software-pipelined ----
            with (
                tc.tile_pool(name="outps", bufs=4, space="PSUM") as ops,
                tc.tile_pool(name="scps", bufs=4, space="PSUM") as sps,
            ):
                outps = {}  # head -> [piece tiles]
                scs = {}  # task -> [sc tiles per piece] / band group tiles
                ets = {}  # matching e^T tiles

                def emit_scores(task, pieces=tuple(range(NPIECE))):
                    h, i, kind = task
                    kc, kr = 3 + h // 2, (h % 2) * D
                    qc, qr = h // 2, (h % 2) * D
                    if kind == "band":
                        chunks = BGROUPS[i]
                        sc = sps.tile([P, SLOTW], f32, tag="sc",
                                      name=f"sb{h}_{i}")
                        for jj, j in enumerate(chunks):
                            nc.tensor.matmul(
                                sc[:, jj * P : (jj + 1) * P],
                                qk[kr : kr + D, kc, j * P : (j + 1) * P],
                                qk[qr : qr + D, qc, j * P : (j + 1) * P],
                                start=(jj == 0),
                                stop=(jj == len(chunks) - 1),
                            )
                        scs[task] = sc
                        return
                    m = CHS[i]
                    if task not in scs:
                        scs[task] = [None] * NPIECE
                    for pi in pieces:
                        qo, qw = QPIECES[pi]
                        sc = sps.tile([P, SLOTW], f32, tag="sc",
                                      name=f"sc{h}_{i}")
                        nc.tensor.matmul(
                            sc[:m, :qw],
                            qk[kr : kr + D, kc, i * P : i * P + m],
                            qk[qr : qr + D, qc, qo : qo + qw],
                            start=True,
                            stop=True,
                        )
                        scs[task][pi] = sc

                def emit_exp(task):
                    h, i, kind = task
                    if kind == "band":
                        chunks = BGROUPS[i]
                        gw = len(chunks) * P
                        c0 = chunks[0] - BSTART
                        et = wp.tile([P, SLOTW], bf16, tag="et",
                                     name=f"eb{h}_{i}")
                        nc.scalar.activation(
                            et[:, :gw], scs[task][:, :gw], AF.Exp, scale=SCALE,
                        )
                        em = wp.tile([P, SLOTW], bf16, tag="et",
                                     name=f"em{h}_{i}")
                        nc.vector.tensor_mul(
                            em[:, :gw],
                            et[:, :gw],
                            dmask[:, c0 : c0 + len(chunks), :].rearrange(
                                "p a b -> p (a b)"
                            ),
                        )
                        ets[task] = em
                        return
                    m = CHS[i]
                    if i < BSTART:
                        bias_ap = 0.0  # all-kept chunk for every batch
                    else:
                        bias_ap = bias[:m, i : i + 1]
                    tiles = []
                    for pi, (qo, qw) in enumerate(QPIECES):
                        et = wp.tile([P, SLOTW], bf16, tag="et",
                                     name=f"et{h}_{i}")
                        nc.scalar.activation(
                            et[:m, :qw],
                            scs[task][pi][:m, :qw],
                            AF.Exp,
                            bias=bias_ap,
                            scale=SCALE,
                        )
                        tiles.append(et)
                    ets[task] = tiles

                def emit_vmm(task, pieces=tuple(range(NPIECE))):
                    h, i, kind = task
                    if kind == "band":
                        em = ets[task]
                        for jj, j in enumerate(BGROUPS[i]):
                            pi = _piece_of_chunk(j)
                            off = j * P - QPIECES[pi][0]
                            w = min(P, QPIECES[pi][1] - off)
                            nc.tensor.matmul(
                                outps[h][pi][:, off : off + w],
                                vaug[:, j, h, :],
                                em[:, jj * P : jj * P + w],
                                start=False,
                                stop=False,
                            )
                        del ets[task], scs[task]
                        return
                    m = CHS[i]
                    if i == 0 and h not in outps:
                        outps[h] = []
                        for _pi in range(NPIECE):
                            o_ = ops.tile([D + 1, SLOTW], f32, tag="outT",
                                          name=f"o{_pi}_{h}")
                            outps[h].append(o_)
                    for pi in pieces:
                        qo, qw = QPIECES[pi]
                        nc.tensor.matmul(
                            outps[h][pi][:, :qw],
                            vaug[:m, i, h, :],
                            ets[task][pi][:m, :qw],
                            start=(i == 0),
                            stop=(i == KC - 1),
                        )
                    if pieces[-1] == NPIECE - 1:
                        del ets[task], scs[task]

                def emit_proj(j):
                    # output projection chunk j (reuses sc PSUM slots)
                    m = CHS[j]
                    yp = sps.tile([P, SLOTW], f32, tag="sc", name=f"yp{j}")
                    for c in range(3):
                        nc.tensor.matmul(
                            yp[:m, :C],
                            ot[:, c, j * P : j * P + m],
                            wproj[:, c, :],
                            start=(c == 0),
                            stop=(c == 2 and not HASB),
                        )
                    if HASB:
                        nc.tensor.matmul(
                            yp[:m, :C],
                            ones[:, :m],
                            bvec[:, :],
                            start=False,
                            stop=True,
                        )
                    ys = wp.tile([P, C], bf16, tag="ys", name=f"ys{j}")
                    nc.vector.tensor_copy(ys[:m, :], yp[:m, :C])
                    nc.sync.dma_start(out_d[j * P : j * P + m, :], ys[:m, :])

                def emit_norm(h, pi):
                    # custom-DVE ops must read SBUF: copy the PSUM denom row
                    # first, broadcast on gpsimd, then recip + mul on DVE.
                    # +EPS dropped: S >= the always-kept diagonal term.
                    qc, qr = h // 2, (h % 2) * D
                    qo, qw = QPIECES[pi]
                    srow = wp.tile([1, SLOTW], f32, tag="srow",
                                   name=f"sr{h}{pi}")
                    nc.vector.tensor_copy(
                        srow[:, :qw], outps[h][pi][D : D + 1, :qw]
                    )
                    rbr = wp.tile([D, SLOTW], f32, tag="rbr", name=f"rr{h}{pi}")
                    nc.gpsimd.partition_broadcast(rbr[:, :qw], srow[:, :qw])
                    rbs = wp.tile([D, SLOTW], f32, tag="rbs", name=f"rs{h}{pi}")
                    nc.vector.reciprocal_approx_fast(rbs[:, :qw], rbr[:, :qw])
                    nc.vector.tensor_mul(
                        ot[qr : qr + D, qc, qo : qo + qw],
                        outps[h][pi][0:D, :qw],
                        rbs[:, :qw],
                    )

                # ---- task list: band tiles interleave early (before the
                # kept KC-1 stop); h0's bands go late (their V chunks and
                # K columns stream in during h0's kept tasks)
                tasks = []
                for h in range(H):
                    kt = [(h, i, "kept") for i in range(KC)]
                    bt = [(h, g, "band") for g in range(len(BGROUPS))]
                    if h == 0:
                        merged = kt[: KC - 1] + bt + kt[KC - 1 :]
                    else:
                        merged = [kt[0]]
                        rest = kt[1 : KC - 1]
                        for gi, b in enumerate(bt):
                            merged.append(b)
                            merged += rest[gi * 2 : gi * 2 + 2]
                        merged += rest[len(bt) * 2 :] + kt[KC - 1 :]
                    tasks += merged
                tindex = {t: i for i, t in enumerate(tasks)}

                # deferred units keyed by task index
                deferred = {}
                # K^T columns beyond piece 0 feed kept chunks 4+ / bands
                for pi_, (qo, qw) in enumerate(_subsplit(0, N)):
                    if pi_ >= 1:
                        deferred.setdefault(pi_ - 1, []).append(
                            _qkv_unit(sps, "sc", 3, qo, qw)
                        )
                # V chunks NV_UP.. : kept ones 2 tasks ahead of their vmm;
                # band-only ones before h0's band tiles
                h0_band_pos = tindex[(0, 0, "band")] if BGROUPS else KC - 1
                for i in range(NV_UP, NCH):
                    if i < KC:
                        t = max(0, i - 2)
                    else:
                        t = max(0, min(i - NCH + h0_band_pos,
                                       h0_band_pos - 1))
                    deferred.setdefault(t, []).append(_v_unit(sps, i))
                # qkv waves: cc1/4 during heads 0-1, cc2/5 during heads 2-3
                per_head = KC + len(BGROUPS)
                for wave, ccs in enumerate(((1, 4), (2, 5))):
                    units = [_qkv_unit(sps, "sc", cc, qo, qw)
                             for cc in ccs for (qo, qw) in _subsplit(0, N)]
                    start = per_head * 2 * wave + (2 if wave == 0 else 0)
                    width = per_head * 2 - 4
                    step = max(1, width // len(units))
                    for u, unit in enumerate(units):
                        deferred.setdefault(start + u * step, []).append(unit)

                # norms: all pieces complete at the kept KC-1 task
                norm_after = {}
                for h in range(H):
                    norm_after[tindex[(h, KC - 1, "kept")]] = [
                        (h, pi) for pi in range(NPIECE)
                    ]

                # minimal upfront: Q^T (cc0) all pieces + K^T piece 0 (covers
                # key chunks 0..3), then the first scores; V chunks 0..2
                _qkv_unit(sps, "sc", 0, 0, 512)()
                _qkv_unit(sps, "sc", 3, 0, 512)()
                for (qo, qw) in _subsplit(0, N)[1:]:
                    _qkv_unit(sps, "sc", 0, qo, qw)()
                emit_scores(tasks[0])
                for i in range(NV_UP):
                    _v_unit(sps, i)()
                for t, task in enumerate(tasks):
                    emit_exp(task)
                    nxt = tasks[t + 1] if t + 1 < len(tasks) else None
                    pieces_norm = norm_after.get(t, [])
                    if (nxt is not None and nxt[2] == "kept"
                            and task[2] == "kept" and not pieces_norm):
                        # fine-grained PE interleave: next scores and current
                        # V-matmuls alternate per query piece
                        for _pi in range(NPIECE):
                            emit_scores(nxt, pieces=(_pi,))
                            emit_vmm(task, pieces=(_pi,))
                    elif pieces_norm and task[2] == "kept":
                        # last kept task of the head: norm (+proj for the
                        # last head) fires per piece as its vmm lands
                        for _pi in range(NPIECE):
                            if nxt is not None and nxt[2] == "kept":
                                emit_scores(nxt, pieces=(_pi,))
                            elif nxt is not None and _pi == 0:
                                emit_scores(nxt)
                            emit_vmm(task, pieces=(_pi,))
                            (h, pi) = pieces_norm[_pi]
                            emit_norm(h, pi)
                            if h == H - 1:
                                qo, qw = QPIECES[pi]
                                for j in range(NCH):
                                    if (qo <= j * P
                                            and j * P + CHS[j] <= qo + qw):
                                        emit_proj(j)
                    else:
                        if nxt is not None:
                            emit_scores(nxt)
                        emit_vmm(task)
                    for unit in deferred.get(t, []):
                        unit()

    nc.finalize()
    return nc


def _prep_core_inputs(x_b, p_b, wqkvT, wprojT, bvec):
    """Permute tokens kept-keys-first; build exp-bias and diag-mask tensors.
    Returns (in_map, perm)."""
    import ml_dtypes

    bf16 = ml_dtypes.bfloat16
    perm = np.argsort(-p_b, kind="stable")
    xT = np.ascontiguousarray(x_b[perm].T).astype(bf16)
    p_perm = p_b[perm].astype(np.float32)
    pad = NCH * P - N
    p_pad = np.concatenate([p_perm, np.ones(pad, np.float32)])
    # bias_exp[r, i] = -30 * (1 - p[i*128 + r]) per key chunk
    bias = (NEG * (1.0 - p_pad)).reshape(NCH, P).T.copy()
    # dmask[:, i, :] = diag(1 - p_chunk_i); pad rows are 0 (p_pad=1)
    dmask = np.zeros((P, NCH, P), np.float32)
    for i in range(NCH):
        chunk = p_pad[i * P : (i + 1) * P]
        np.fill_diagonal(dmask[:, i, :], 1.0 - chunk)
    return {
        "xT": xT,
        "wqkvT": wqkvT,
        "wprojT": wprojT,
        "bias_exp": np.ascontiguousarray(bias),
        "dmask": dmask.astype(bf16),
        "bvec": bvec,
    }, perm


def _install_ntff_hook():
    """The container's antenv package lacks axon_hooks; recreate the NTFF
    profile hook (mirrors trn_agent_boot) so trace=True yields exec_time."""
    import types
    import ctypes
    import contextlib

    if "antenv.axon_hooks" in sys.modules:
        return
    so_path = "/opt/axon/libaxon_pjrt.so"
    mod = types.ModuleType("antenv.axon_hooks")
    state = {"hook": None}
    mod.set_axon_ntff_profile_hook = lambda h: state.__setitem__("hook", h)
    mod.get_axon_ntff_profile_hook = lambda: state["hook"]
    sys.modules["antenv.axon_hooks"] = mod

    try:
        lib = ctypes.CDLL(so_path)
    except OSError:
        return
    if not hasattr(lib, "axon_start_nrt_profile"):
        return
    lib.axon_start_nrt_profile.argtypes = [
        ctypes.POINTER(ctypes.c_int64),
        ctypes.c_size_t,
    ]
    lib.axon_start_nrt_profile.restype = ctypes.c_int64
    lib.axon_stop_nrt_profile.argtypes = [ctypes.c_char_p]
    lib.axon_stop_nrt_profile.restype = ctypes.c_int64

    @contextlib.contextmanager
    def _hook(output_dir, device_ids):
        import jax

        jax.devices()
        if device_ids:
            ids = (ctypes.c_int64 * len(device_ids))(*device_ids)
            rc = lib.axon_start_nrt_profile(ids, len(device_ids))
        else:
            rc = lib.axon_start_nrt_profile(None, 0)
        if rc != 0:
            raise RuntimeError(f"axon_start_nrt_profile rc={rc}")
        try:
            yield
        finally:
            n = lib.axon_stop_nrt_profile(str(output_dir).encode())
            print(f"profile: {n} file(s) written to {output_dir}", file=sys.stderr)

    state["hook"] = _hook


def kernel(x, vis_tube, w_qkv, w_proj, b_proj, _trace=False):
    from concourse.bass_utils import run_bass_kernel_spmd

    import ml_dtypes

    if _trace:
        _install_ntff_hook()

    bf16 = ml_dtypes.bfloat16
    x = np.asarray(x, np.float32)
    p = np.asarray(vis_tube, np.float32)[:, :, 0]
    keeps = (p > 0.5).sum(axis=1)  # kept keys per batch
    KC = max(1, int(-(-keeps.max() // P)))  # chunks containing kept keys
    BSTART = int(keeps.min() // P)  # first chunk containing a dropped key

    HASB = bool(np.any(np.asarray(b_proj)))
    key = (KC, BSTART, HASB)
    if _CACHE.get("key") != key:
        _CACHE["nc"] = _build_nc(KC, BSTART, HASB)
        _CACHE["key"] = key
    nc = _CACHE["nc"]

    wqkvT = np.ascontiguousarray(np.asarray(w_qkv).T).astype(bf16)
    wprojT = np.ascontiguousarray(np.asarray(w_proj).T).astype(bf16)
    bvec = np.asarray(b_proj).reshape(1, C).astype(np.float32).astype(bf16)
    in_maps, perms = [], []
    for b in range(B):
        im, perm = _prep_core_inputs(x[b], p[b], wqkvT, wprojT, bvec)
        in_maps.append(im)
        perms.append(perm)
    res = run_bass_kernel_spmd(nc, in_maps, core_ids=list(range(B)), trace=_trace)
    out = np.empty((B, N, C), np.float32)
    for b in range(B):
        out[b][perms[b]] = np.asarray(res.results[b]["out"], np.float32)
    if _trace:
        _CACHE["last_result"] = res
    return out


# revision 72
# speedup vs baseline: 1.0153x; 1.0153x over previous
"""Trainium2 Bass kernel for policy-masked attention (nn_Attention_5007931867377).

Reference computation (per batch b):
    qkv = x @ w_qkv.T ; split into q,k,v heads [H=6, N=1568, D=64]
    s   = (q @ k.T) * D**-0.5
    mask[m] visibility per key + diagonal always kept
    e   = exp(s - max) * mask ; attn = (e + EPS/N)/(sum e + EPS)
    out = (attn @ v) concat heads @ w_proj.T + b_proj

Strategy: pure data parallel, one batch element per NeuronCore (8 cores).
Per-core dataflow (everything transposed so softmax reductions are on the
free axis and e^T feeds the V-matmul without on-chip transposes):
  - host passes x^T (bf16); on chip: Q^T,K^T in [c_out, n] layout, V in [n, d]
  - tokens permuted host-side so kept keys come first; KC chunks of 128
    keys run the full path, chunks BSTART.. are dropped keys, visible only
    to their own query (diagonal)
  - scores^T[m, q] = K^T.T @ Q^T  (PSUM, per 128-key chunk x query piece)
  - key-mask folded into exp as per-partition bias (-30 * (1-p))
  - diagonal-keep for dropped keys: per-head band tiles batch FOUR 128x128
    diagonal blocks into one PSUM bank; one 512-wide exp (no bias), then a
    DVE mul with a host-built diag(1-p) mask keeps exactly the wanted
    diagonal entries.  qk is padded to 13*128 columns (zeroed) so the last
    chunk is uniform.  Band vmms run before the last kept chunk so kept
    chunk KC-1 uniformly carries the accumulation stop.
  - e^T = exp(0.125*s + bias) ACT -> SBUF (bf16)
  - V augmented with a ones column: outT_aug[65, q] = V_aug.T @ e^T gives
    attention output rows 0..63 and the softmax denominator in row 64
  - normalize: DVE copy of the denominator row, gpsimd partition_broadcast,
    DVE reciprocal_approx_fast, DVE multiply (all-SBUF except the copy;
    custom-DVE ops must NOT read PSUM directly - wrong results on HW)
  - proj: y[n, :] = oT.T @ w_proj.T (+ bias via K=1 ones matmul), DVE copy
    to bf16, DMA out bf16 (host upcasts).

Schedule: xt/w_qkv DMAs are column-split so the first scores' inputs land
first; only qkv cc0/cc3 and V chunks 0..5 run up front, V chunks 6.. and
qkv cc1/4 (heads 2,3) / cc2/5 (heads 4,5) interleave into the attention
stream as PSUM-slot units filling exp-wait gaps.

All matmuls bf16: float32r matmuls don't register as PE activity for
the HAM clock gate and their 4-byte fused weight loads serialize; bf16
runs warm at 2.4 GHz. Max-subtraction is dropped: scores ~ N(0,1).
"""

import sys

if "/opt/trn_rl_repo" not in sys.path:
    sys.path.insert(0, "/opt/trn_rl_repo")

import numpy as np

B, N, C, H = 8, 1568, 384, 6
D = C // H  # 64
SCALE = D ** -0.5  # 0.125
EPS = 1e-6
NEG = -30.0  # masked-key exp bias (exp(-30) ~ 9e-14, way below EPS/N)

P = 128
NCH = (N + P - 1) // P  # 13 key/token chunks (12 x 128 + 1 x 32)
CHS = [min(P, N - i * P) for i in range(NCH)]
NPAD = NCH * P  # 1664: qk padded so band chunks are uniformly 128 wide
# query pieces: 128-aligned, <=1 PSUM bank each
QPIECES = [(0, 512), (512, 512), (1024, 512), (1536, 32)]
SLOTW = 512
NPIECE = len(QPIECES)


def _subsplit(qo, qw, step=512):
    return [(qo + o, min(step, qw - o)) for o in range(0, qw, step)]


def _piece_of_chunk(j):
    for pi, (qo, qw) in enumerate(QPIECES):
        if qo <= j * P and j * P + CHS[j] <= qo + qw:
            return pi
    raise AssertionError


_CACHE = {}


def _build_nc(KC, BSTART, HASB):
    import concourse.tile as tile
    from concourse import bacc, mybir

    dt = mybir.dt
    f32 = dt.float32
    bf16 = dt.bfloat16
    AF = mybir.ActivationFunctionType

    NB = NCH - BSTART  # number of diagonal band chunks
    BGROUPS = []  # band chunk groups: 4 blocks fill one PSUM bank
    _b = list(range(BSTART, NCH))
    for g0 in range(0, NB, 4):
        BGROUPS.append(_b[g0 : g0 + 4])

    nc = bacc.Bacc()

    xT_d = nc.declare_dram_parameter("xT", [C, N], bf16, isOutput=False)
    wqkvT_d = nc.declare_dram_parameter("wqkvT", [C, 3 * C], bf16, isOutput=False)
    wprojT_d = nc.declare_dram_parameter("wprojT", [C, C], bf16, isOutput=False)
    bias_d = nc.declare_dram_parameter("bias_exp", [P, NCH], f32, isOutput=False)
    dmask_d = nc.declare_dram_parameter("dmask", [P, NCH, P], bf16, isOutput=False)
    bvec_d = nc.declare_dram_parameter("bvec", [1, C], bf16, isOutput=False)
    out_d = nc.declare_dram_parameter("out", [N, C], bf16, isOutput=True)

    with tile.TileContext(nc, pool_alloc_mode="queue") as tc:
        with (
            tc.tile_pool(name="persist", bufs=1) as pp,
            tc.tile_pool(name="work", bufs=6) as wp,
        ):
            # ---- persistent SBUF tensors ----
            xt = pp.tile([P, 3, N], bf16, tag="xt")  # x^T chunks (c rows)
            wqkv = pp.tile([P, 3, 3 * C], bf16, tag="wqkv")
            wproj = pp.tile([P, 3, C], bf16, tag="wproj")
            qk = pp.tile([P, 6, NPAD], bf16, tag="qk")  # Q^T(0..2), K^T(3..5)
            vaug = pp.tile([P, NCH, H, D + 1], bf16, tag="vaug")
            ot = pp.tile([P, 3, N], bf16, tag="ot")  # normalized attn out ^T
            bias = pp.tile([P, NCH], f32, tag="bias")
            dmask = pp.tile([P, NB, P], bf16, tag="dmask")
            bvec = pp.tile([1, C], bf16, tag="bvec")
            ones = pp.tile([1, P], bf16, tag="ones")
            warm = pp.tile([1, 1], f32, tag="warm")

            # split big input DMAs per c-chunk so they round-robin across
            # DMA queues (v1 ordering: keeps the dense upfront matmul block
            # fed, which in turn keeps the HAM clock governor at full speed)
            xr = xT_d[:].rearrange("(a p) n -> p a n", p=P)
            qr_ = wqkvT_d[:].rearrange("(a p) n -> p a n", p=P)
            pr = wprojT_d[:].rearrange("(a p) n -> p a n", p=P)
            for c in range(3):
                nc.sync.dma_start(xt[:, c, :], xr[:, c, :])
                nc.sync.dma_start(wqkv[:, c, :], qr_[:, c, :])
            for c in range(3):
                nc.sync.dma_start(wproj[:, c, :], pr[:, c, :])
            nc.sync.dma_start(bias[:, :], bias_d[:])
            nc.sync.dma_start(dmask[:, :, :], dmask_d[:, BSTART:NCH, :])
            nc.sync.dma_start(bvec[:, :], bvec_d[:])
            nc.vector.memset(ones[:, :], 1.0)
            nc.vector.memset(warm[:, :], NEG)
            # dummy exp so the ACT table set loads during the DMA prologue
            nc.scalar.activation(warm[:, :], warm[:, :], AF.Exp)
            # zero the pad: band chunk NCH-1 runs 128 wide (pad scores
            # exp(0)=1 are killed by dmask; vaug pad rows contribute 0)
            nc.vector.memset(qk[:, :, N:NPAD], 0.0)
            nc.vector.memset(vaug[:, NCH - 1, :, :], 0.0)

            # ---- phase 1: minimal upfront qkv ----
            def _qkv_unit(pool, tag, cc, qo, qw):
                def emit():
                    ps = pool.tile([P, 512], f32, tag=tag,
                                   name=f"qp{cc}_{qo}")
                    for c in range(3):
                        nc.tensor.matmul(
                            ps[:, :qw],
                            wqkv[:, c, cc * P : (cc + 1) * P],
                            xt[:, c, qo : qo + qw],
                            start=(c == 0),
                            stop=(c == 2),
                        )
                    nc.vector.tensor_copy(qk[:, cc, qo : qo + qw], ps[:, :qw])
                return emit

            def _v_unit(pool, i):
                def emit():
                    m = CHS[i]
                    ps = pool.tile([P, 512], f32, tag="sc", name=f"v{i}")
                    for c in range(3):
                        nc.tensor.matmul(
                            ps[:m, :C],
                            xt[:, c, i * P : i * P + m],
                            wqkv[:, c, 2 * C : 3 * C],
                            start=(c == 0),
                            stop=(c == 2),
                        )
                    nc.vector.tensor_copy(
                        vaug[:m, i, :, 0:D],
                        ps[:m, :C].rearrange("p (h d) -> p h d", h=H),
                    )
                    nc.vector.memset(vaug[:m, i, :, D : D + 1], 1.0)
                return emit

            # ---- phase 1: dense upfront qkv (cc0/cc3 + all V chunks).
            # This back-to-back matmul block ramps and HOLDS the HAM clock
            # governor at full speed; deferring it (sparser PE stream)
            # measured ~2x more time spent throttled at half clock.
            with tc.tile_pool(name="qkvps", bufs=3, space="PSUM") as qps:
                for cc in (0, 3):
                    for (qo, qw) in _subsplit(0, N):
                        _qkv_unit(qps, "qk", cc, qo, qw)()
                for i in range(NCH):
                    _v_unit(qps, i)()

            # ---- phase 2: attention, software-pipelined ----
            with (
                tc.tile_pool(name="outps", bufs=4, space="PSUM") as ops,
                tc.tile_pool(name="scps", bufs=4, space="PSUM") as sps,
            ):
                outps = {}  # head -> [piece tiles]
                scs = {}  # task -> [sc tiles per piece] / band group tiles
                ets = {}  # matching e^T tiles

                def emit_scores(task, pieces=tuple(range(NPIECE))):
                    h, i, kind = task
                    kc, kr = 3 + h // 2, (h % 2) * D
                    qc, qr = h // 2, (h % 2) * D
                    if kind == "band":
                        chunks = BGROUPS[i]
                        sc = sps.tile([P, SLOTW], f32, tag="sc",
                                      name=f"sb{h}_{i}")
                        for jj, j in enumerate(chunks):
                            nc.tensor.matmul(
                                sc[:, jj * P : (jj + 1) * P],
                                qk[kr : kr + D, kc, j * P : (j + 1) * P],
                                qk[qr : qr + D, qc, j * P : (j + 1) * P],
                                start=(jj == 0),
                                stop=(jj == len(chunks) - 1),
                            )
                        scs[task] = sc
                        return
                    m = CHS[i]
                    if task not in scs:
                        scs[task] = [None] * NPIECE
                    for pi in pieces:
                        qo, qw = QPIECES[pi]
                        sc = sps.tile([P, SLOTW], f32, tag="sc",
                                      name=f"sc{h}_{i}")
                        nc.tensor.matmul(
                            sc[:m, :qw],
                            qk[kr : kr + D, kc, i * P : i * P + m],
                            qk[qr : qr + D, qc, qo : qo + qw],
                            start=True,
                            stop=True,
                        )
                        scs[task][pi] = sc

                def emit_exp(task):
                    h, i, kind = task
                    if kind == "band":
                        chunks = BGROUPS[i]
                        gw = len(chunks) * P
                        c0 = chunks[0] - BSTART
                        et = wp.tile([P, SLOTW], bf16, tag="et",
                                     name=f"eb{h}_{i}")
                        nc.scalar.activation(
                            et[:, :gw], scs[task][:, :gw], AF.Exp, scale=SCALE,
                        )
                        em = wp.tile([P, SLOTW], bf16, tag="et",
                                     name=f"em{h}_{i}")
                        nc.vector.tensor_mul(
                            em[:, :gw],
                            et[:, :gw],
                            dmask[:, c0 : c0 + len(chunks), :].rearrange(
                                "p a b -> p (a b)"
                            ),
                        )
                        ets[task] = em
                        return
                    m = CHS[i]
                    if i < BSTART:
                        bias_ap = 0.0  # all-kept chunk for every batch
                    else:
                        bias_ap = bias[:m, i : i + 1]
                    tiles = []
                    for pi, (qo, qw) in enumerate(QPIECES):
                        et = wp.tile([P, SLOTW], bf16, tag="et",
                                     name=f"et{h}_{i}")
                        nc.scalar.activation(
                            et[:m, :qw],
                            scs[task][pi][:m, :qw],
                            AF.Exp,
                            bias=bias_ap,
                            scale=SCALE,
                        )
                        tiles.append(et)
                    ets[task] = tiles

                def emit_vmm(task, pieces=tuple(range(NPIECE))):
                    h, i, kind = task
                    if kind == "band":
                        em = ets[task]
                        for jj, j in enumerate(BGROUPS[i]):
                            pi = _piece_of_chunk(j)
                            off = j * P - QPIECES[pi][0]
                            w = min(P, QPIECES[pi][1] - off)
                            nc.tensor.matmul(
                                outps[h][pi][:, off : off + w],
                                vaug[:, j, h, :],
                                em[:, jj * P : jj * P + w],
                                start=False,
                                stop=False,
                            )
                        del ets[task], scs[task]
                        return
                    m = CHS[i]
                    if i == 0 and h not in outps:
                        outps[h] = []
                        for _pi in range(NPIECE):
                            o_ = ops.tile([D + 1, SLOTW], f32, tag="outT",
                                          name=f"o{_pi}_{h}")
                            outps[h].append(o_)
                    for pi in pieces:
                        qo, qw = QPIECES[pi]
                        nc.tensor.matmul(
                            outps[h][pi][:, :qw],
                            vaug[:m, i, h, :],
                            ets[task][pi][:m, :qw],
                            start=(i == 0),
                            stop=(i == KC - 1),
                        )
                    if pieces[-1] == NPIECE - 1:
                        del ets[task], scs[task]

                def emit_proj(j):
                    # output projection chunk j (reuses sc PSUM slots)
                    m = CHS[j]
                    yp = sps.tile([P, SLOTW], f32, tag="sc", name=f"yp{j}")
                    for c in range(3):
                        nc.tensor.matmul(
                            yp[:m, :C],
                            ot[:, c, j * P : j * P + m],
                            wproj[:, c, :],
                            start=(c == 0),
                            stop=(c == 2 and not HASB),
                        )
                    if HASB:
                        nc.tensor.matmul(
                            yp[:m, :C],
                            ones[:, :m],
                            bvec[:, :],
                            start=False,
                            stop=True,
                        )
                    ys = wp.tile([P, C], bf16, tag="ys", name=f"ys{j}")
                    nc.vector.tensor_copy(ys[:m, :], yp[:m, :C])
                    nc.sync.dma_start(out_d[j * P : j * P + m, :], ys[:m, :])

                def emit_norm(h, pi):
                    # custom-DVE ops must read SBUF: copy the PSUM denom row
                    # first, broadcast on gpsimd, then recip + mul on DVE.
                    # +EPS dropped: S >= the always-kept diagonal term.
                    qc, qr = h // 2, (h % 2) * D
                    qo, qw = QPIECES[pi]
                    srow = wp.tile([1, SLOTW], f32, tag="srow",
                                   name=f"sr{h}{pi}")
                    nc.vector.tensor_copy(
                        srow[:, :qw], outps[h][pi][D : D + 1, :qw]
                    )
                    rbr = wp.tile([D, SLOTW], f32, tag="rbr", name=f"rr{h}{pi}")
                    nc.gpsimd.partition_broadcast(rbr[:, :qw], srow[:, :qw])
                    rbs = wp.tile([D, SLOTW], f32, tag="rbs", name=f"rs{h}{pi}")
                    nc.vector.reciprocal_approx_fast(rbs[:, :qw], rbr[:, :qw])
                    nc.vector.tensor_mul(
                        ot[qr : qr + D, qc, qo : qo + qw],
                        outps[h][pi][0:D, :qw],
                        rbs[:, :qw],
                    )

                # ---- task list: band tiles interleave between early kept
                # tasks (always before the kept KC-1 stop matmuls)
                tasks = []
                for h in range(H):
                    kt = [(h, i, "kept") for i in range(KC)]
                    bt = [(h, g, "band") for g in range(len(BGROUPS))]
                    merged = [kt[0]]
                    rest = kt[1 : KC - 1]
                    for gi, b in enumerate(bt):
                        merged.append(b)
                        merged += rest[gi * 2 : gi * 2 + 2]
                    merged += rest[len(bt) * 2 :] + kt[KC - 1 :]
                    tasks += merged
                tindex = {t: i for i, t in enumerate(tasks)}

                # deferred units keyed by task index
                deferred = {}
                # qkv waves: cc1/4 during heads 0-1, cc2/5 during heads 2-3
                per_head = KC + len(BGROUPS)
                for wave, ccs in enumerate(((1, 4), (2, 5))):
                    units = [_qkv_unit(sps, "sc", cc, qo, qw)
                             for cc in ccs for (qo, qw) in _subsplit(0, N)]
                    start = per_head * 2 * wave + (2 if wave == 0 else 0)
                    width = per_head * 2 - 4
                    step = max(1, width // len(units))
                    for u, unit in enumerate(units):
                        deferred.setdefault(start + u * step, []).append(unit)

                # norms: all pieces complete at the kept KC-1 task
                norm_after = {}
                for h in range(H):
                    norm_after[tindex[(h, KC - 1, "kept")]] = [
                        (h, pi) for pi in range(NPIECE)
                    ]

                emit_scores(tasks[0])
                for t, task in enumerate(tasks):
                    emit_exp(task)
                    nxt = tasks[t + 1] if t + 1 < len(tasks) else None
                    pieces_norm = norm_after.get(t, [])
                    if (nxt is not None and nxt[2] == "kept"
                            and task[2] == "kept" and not pieces_norm):
                        # fine-grained PE interleave: next scores and current
                        # V-matmuls alternate per query piece
                        for _pi in range(NPIECE):
                            emit_scores(nxt, pieces=(_pi,))
                            emit_vmm(task, pieces=(_pi,))
                    elif pieces_norm and task[2] == "kept":
                        # last kept task of the head: norm (+proj for the
                        # last head) fires per piece as its vmm lands
                        for _pi in range(NPIECE):
                            if nxt is not None and nxt[2] == "kept":
                                emit_scores(nxt, pieces=(_pi,))
                            elif nxt is not None and _pi == 0:
                                emit_scores(nxt)
                            emit_vmm(task, pieces=(_pi,))
                            (h, pi) = pieces_norm[_pi]
                            emit_norm(h, pi)
                            if h == H - 1:
                                qo, qw = QPIECES[pi]
                                for j in range(NCH):
                                    if (qo <= j * P
                                            and j * P + CHS[j] <= qo + qw):
                                        emit_proj(j)
                    else:
                        if nxt is not None:
                            emit_scores(nxt)
                        emit_vmm(task)
                    for unit in deferred.get(t, []):
                        unit()

    nc.finalize()
    return nc


def _prep_core_inputs(x_b, p_b, wqkvT, wprojT, bvec):
    """Permute tokens kept-keys-first; build exp-bias and diag-mask tensors.
    Returns (in_map, perm)."""
    import ml_dtypes

    bf16 = ml_dtypes.bfloat16
    perm = np.argsort(-p_b, kind="stable")
    xT = np.ascontiguousarray(x_b[perm].T).astype(bf16)
    p_perm = p_b[perm].astype(np.float32)
    pad = NCH * P - N
    p_pad = np.concatenate([p_perm, np.ones(pad, np.float32)])
    # bias_exp[r, i] = -30 * (1 - p[i*128 + r]) per key chunk
    bias = (NEG * (1.0 - p_pad)).reshape(NCH, P).T.copy()
    # dmask[:, i, :] = diag(1 - p_chunk_i); pad rows are 0 (p_pad=1)
    dmask = np.zeros((P, NCH, P), np.float32)
    for i in range(NCH):
        chunk = p_pad[i * P : (i + 1) * P]
        np.fill_diagonal(dmask[:, i, :], 1.0 - chunk)
    return {
        "xT": xT,
        "wqkvT": wqkvT,
        "wprojT": wprojT,
        "bias_exp": np.ascontiguousarray(bias),
        "dmask": dmask.astype(bf16),
        "bvec": bvec,
    }, perm


def _install_ntff_hook():
    """The container's antenv package lacks axon_hooks; recreate the NTFF
    profile hook (mirrors trn_agent_boot) so trace=True yields exec_time."""
    import types
    import ctypes
    import contextlib

    if "antenv.axon_hooks" in sys.modules:
        return
    so_path = "/opt/axon/libaxon_pjrt.so"
    mod = types.ModuleType("antenv.axon_hooks")
    state = {"hook": None}
    mod.set_axon_ntff_profile_hook = lambda h: state.__setitem__("hook", h)
    mod.get_axon_ntff_profile_hook = lambda: state["hook"]
    sys.modules["antenv.axon_hooks"] = mod

    try:
        lib = ctypes.CDLL(so_path)
    except OSError:
        return
    if not hasattr(lib, "axon_start_nrt_profile"):
        return
    lib.axon_start_nrt_profile.argtypes = [
        ctypes.POINTER(ctypes.c_int64),
        ctypes.c_size_t,
    ]
    lib.axon_start_nrt_profile.restype = ctypes.c_int64
    lib.axon_stop_nrt_profile.argtypes = [ctypes.c_char_p]
    lib.axon_stop_nrt_profile.restype = ctypes.c_int64

    @contextlib.contextmanager
    def _hook(output_dir, device_ids):
        import jax

        jax.devices()
        if device_ids:
            ids = (ctypes.c_int64 * len(device_ids))(*device_ids)
            rc = lib.axon_start_nrt_profile(ids, len(device_ids))
        else:
            rc = lib.axon_start_nrt_profile(None, 0)
        if rc != 0:
            raise RuntimeError(f"axon_start_nrt_profile rc={rc}")
        try:
            yield
        finally:
            n = lib.axon_stop_nrt_profile(str(output_dir).encode())
            print(f"profile: {n} file(s) written to {output_dir}", file=sys.stderr)

    state["hook"] = _hook


def kernel(x, vis_tube, w_qkv, w_proj, b_proj, _trace=False):
    from concourse.bass_utils import run_bass_kernel_spmd

    import ml_dtypes

    if _trace:
        _install_ntff_hook()

    bf16 = ml_dtypes.bfloat16
    x = np.asarray(x, np.float32)
    p = np.asarray(vis_tube, np.float32)[:, :, 0]
    keeps = (p > 0.5).sum(axis=1)  # kept keys per batch
    KC = max(1, int(-(-keeps.max() // P)))  # chunks containing kept keys
    BSTART = int(keeps.min() // P)  # first chunk containing a dropped key

    HASB = bool(np.any(np.asarray(b_proj)))
    key = (KC, BSTART, HASB)
    if _CACHE.get("key") != key:
        _CACHE["nc"] = _build_nc(KC, BSTART, HASB)
        _CACHE["key"] = key
    nc = _CACHE["nc"]

    wqkvT = np.ascontiguousarray(np.asarray(w_qkv).T).astype(bf16)
    wprojT = np.ascontiguousarray(np.asarray(w_proj).T).astype(bf16)
    bvec = np.asarray(b_proj).reshape(1, C).astype(np.float32).astype(bf16)
    in_maps, perms = [], []
    for b in range(B):
        im, perm = _prep_core_inputs(x[b], p[b], wqkvT, wprojT, bvec)
        in_maps.append(im)
        perms.append(perm)
    res = run_bass_kernel_spmd(nc, in_maps, core_ids=list(range(B)), trace=_trace)
    out = np.empty((B, N, C), np.float32)
    for b in range(B):
        out[b][perms[b]] = np.asarray(res.results[b]["out"], np.float32)
    if _trace:
        _CACHE["last_result"] = res
    return out
